# revision 2
# baseline (speedup 1.0000x reference)
"""Trainium2 Bass kernel for nn_Neighbor_Mean (gnn message passing) — v5.

Math: out[b,s,:] = mean_n( mask[b,s,n] * (T_b[idx[b,s,n]] @ Wn^T) )
 with T_b[v] = pos_table[v] + (h[b][v-1] if v>=1 else 0)
Linearity: out[b,s,:] = ( (1/N) * sum_n T'_b[idx_eff[b,s,n]] ) @ Wn^T
 where T' has a zero SINK row at v=2049 and idx_eff = mask ? idx : SINK.

Sharding: data-parallel over batch, one NeuronCore per batch row (B == 8).

v6 = v4's fast natural-wrap index prep + PE-side n-reduction with flipped
matmul operands (walrus rejects multi-free-dim WEIGHTS APs, but the MOVING
tensor AP can be multi-dim):
 - gather order j = 512*g16 + 16*n + s16 (natural wrap -> 128B-descriptor
   index loads, ~20us prologue vs v1's 185us).
 - n-sum on the PE via PSUM accumulation: per 128-s chunk (8 gather calls),
   32 matmuls with lhsT = Wn^T/N (stationary, 1 free dim) and rhs = the
   3-level strided view [h][g16 x8 stride 512][s16 x16 stride 1] selecting
   the 128 same-n columns -> psum[k, s-cols] accumulates over n. A PE
   transpose per chunk flips [k, s] -> [s, k] for the output.
 - The PE reads through its own SBUF port, so the gather ucode's index
   stream (Pool/DVE shared port) is NOT slowed -- v4's DVE tensor_reduce
   cost ~670ns/call of port contention. DVE idles after the prologue.
"""
import sys

sys.path.insert(0, '/opt/trn_rl_repo')

import numpy as np

import concourse.bacc as bacc
import concourse.bass as bass
import concourse.mybir as mybir
import concourse.tile as tile
from concourse.bass_utils import run_bass_kernel_spmd
from concourse.masks import make_identity

B, N, H = 8, 32, 128
NI = 512             # idxs per dma_gather call (HW desc-ring ceiling)
SCHUNK = 128         # s rows per matmul chunk
TPR = 128            # sbuf gather tokens per rank
F32 = mybir.dt.float32
I32 = mybir.dt.int32
I16 = mybir.dt.int16
BF16 = mybir.dt.bfloat16


def build_program(S: int = 2048):
    VPOS = S + 1                      # pos_table rows; SINK index == VPOS
    NRANKS = (VPOS + 1 + 127) // 128  # packed table chunks incl. sink row
    nchunk = S // SCHUNK              # matmul chunks (16)
    cpc = SCHUNK * N // NI            # gather calls per chunk (8)

    nc = bacc.Bacc("TRN2", debug=False, num_swdge_queues=4)
    h_d = nc.dram_tensor("h", [S, H], F32, kind="ExternalInput")
    idx_d = nc.dram_tensor("idx", [S, N], I32, kind="ExternalInput")
    msk_d = nc.dram_tensor("msk", [S, N], I32, kind="ExternalInput")
    pos_d = nc.dram_tensor("pos", [VPOS, H], F32, kind="ExternalInput")
    wn_d = nc.dram_tensor("wn", [H, H], F32, kind="ExternalInput")
    out_d = nc.dram_tensor("out", [S, H], F32, kind="ExternalOutput")

    with tile.TileContext(nc) as tc:
        with (
            tc.tile_pool(name="const", bufs=1) as constp,
            tc.tile_pool(name="stage", bufs=3) as stagep,
            tc.tile_pool(name="idxp", bufs=1) as idxp,
            tc.tile_pool(name="gbig", bufs=2) as gbigp,
            tc.tile_pool(name="outp", bufs=4) as outp,
            tc.tile_pool(name="psum_w", bufs=1, space="PSUM") as psumw,
            tc.tile_pool(name="psum_m", bufs=3, space="PSUM") as psumm,
            tc.tile_pool(name="psum_t", bufs=2, space="PSUM") as psumt,
        ):
            # ---- masked indices, natural wrap (prologue) -------------
            # ucode: gather position j reads idx at (p=j%16, col=j//16).
            # j = 512*g16 + 16*n + s16: wrapped layout == natural:
            # idxw[p, 32*cs + n] = idx[16*cs + p, n].
            # IMPORTANT (from v1): all 2-read DVE ops (copy_predicated) stay
            # in the prologue; every gather transitively depends on idxbuf.
            acols = S * N // 16
            idxw32 = idxp.tile([16, acols], I32, tag="idxw32")
            mskw32 = idxp.tile([16, acols], I32, tag="mskw32")
            idxe32 = idxp.tile([16, acols], I32, tag="idxe32")
            half = S // 2
            nc.sync.dma_start(
                idxw32[:, :acols // 2].rearrange("p (c n) -> p c n", n=N),
                idx_d[:half].rearrange("(c p) n -> p c n", p=16),
            )
            nc.scalar.dma_start(
                idxw32[:, acols // 2:].rearrange("p (c n) -> p c n", n=N),
                idx_d[half:].rearrange("(c p) n -> p c n", p=16),
            )
            nc.sync.dma_start(
                mskw32[:, :acols // 2].rearrange("p (c n) -> p c n", n=N),
                msk_d[:half].rearrange("(c p) n -> p c n", p=16),
            )
            nc.scalar.dma_start(
                mskw32[:, acols // 2:].rearrange("p (c n) -> p c n", n=N),
                msk_d[half:].rearrange("(c p) n -> p c n", p=16),
            )
            nc.vector.memset(idxe32[:], float(VPOS))
            nc.vector.copy_predicated(idxe32[:], mskw32[:], idxw32[:])
            # int32 -> int16 (values <= 2049: take low halves)
            idxbuf = idxp.tile([128, acols], I16, tag="idxbuf")
            lo = idxe32[:].bitcast(I16).rearrange("p (e two) -> p e two", two=2)
            nc.vector.tensor_copy(
                idxbuf[0:16, :].rearrange("p (e one) -> p e one", one=1),
                lo[:, :, 0:1],
            )
            # replicate to the 8 16-partition groups (log doubling)
            nc.sync.dma_start(idxbuf[16:32, :], idxbuf[0:16, :])
            nc.sync.dma_start(idxbuf[32:64, :], idxbuf[0:32, :])
            nc.sync.dma_start(idxbuf[64:128, :], idxbuf[0:64, :])

            # ---- Wn^T * (1/N) in bf16 --------------------------------
            wn_sb = constp.tile([H, H], F32)
            nc.sync.dma_start(wn_sb[:], wn_d[:])
            ident = constp.tile([128, 128], F32)
            make_identity(nc, ident[:])
            wnt_ps = psumw.tile([128, H], F32, tag="wnt")
            nc.tensor.transpose(out=wnt_ps[:], in_=wn_sb[:], identity=ident[:])
            wnt = constp.tile([H, H], BF16)
            nc.vector.tensor_scalar_mul(wnt[:], wnt_ps[:], 1.0 / N)

            # ---- fused table T' (bf16, SWDGE packed layout) ----------
            # tbl[p, q*H:(q+1)*H] = T'[q*128 + p, :]
            tbl = constp.tile([128, NRANKS * H], BF16)
            for q in range(NRANKS):
                v0 = q * 128
                n_pos = min(128, VPOS - v0)       # valid pos rows this chunk
                if n_pos <= 0:
                    nc.vector.memset(tbl[:, q * H:(q + 1) * H], 0.0)
                    continue
                pstage = stagep.tile([128, H], F32, tag="pstage")
                hstage = stagep.tile([128, H], F32, tag="hstage")
                if n_pos < 128:
                    nc.vector.memset(tbl[:, q * H:(q + 1) * H], 0.0)
                eng = nc.sync if q % 2 == 0 else nc.scalar
                eng.dma_start(pstage[:n_pos, :], pos_d[v0:v0 + n_pos, :])
                if q == 0:
                    nc.vector.memset(hstage[0:1, :], 0.0)
                    eng.dma_start(hstage[1:n_pos, :], h_d[0:n_pos - 1, :])
                else:
                    eng.dma_start(hstage[:n_pos, :], h_d[v0 - 1:v0 + n_pos - 1, :])
                nc.vector.tensor_add(
                    tbl[:n_pos, q * H:(q + 1) * H], pstage[:n_pos, :], hstage[:n_pos, :]
                )

            # ---- gather / matmul pipeline ----------------------------
            for t in range(nchunk):
                gbig = gbigp.tile([128, 1, SCHUNK * N], BF16, tag="gbig")
                for c in range(cpc):
                    wc0 = (t * cpc + c) * (NI // 16)
                    nc.gpsimd.dma_gather(
                        gbig[:, :, c * NI:(c + 1) * NI],
                        tbl[:],
                        idxbuf[:, wc0:wc0 + NI // 16],
                        NI, NI, H,
                        transpose=True,
                        queue_num=c % 4,
                        sbuf_tokens_per_rank=TPR,
                        sbuf_free_dim_per_rank=H * 2,
                    )
                # n-sum on PE: cols j = 512*g + 16*n + s16; moving-tensor
                # slice for n = [h][g x8 stride 512][s16 x16 stride 1];
                # psum[k, s-cols] accumulates over n, cols in s order.
                gv = gbig[:, 0, :].rearrange(
                    "p (g n s) -> p n g s", g=cpc, n=N, s=16
                )
                ps = psumm.tile([128, SCHUNK], F32, tag="mm")
                for n in range(N):
                    nc.tensor.matmul(
                        out=ps[:],
                        lhsT=wnt[:],
                        rhs=gv[:, n, :, :],
                        start=(n == 0),
                        stop=(n == N - 1),
                    )
                # [k, s] -> [s, k]: copy to SBUF, PE transpose, copy, DMA
                msb = outp.tile([128, SCHUNK], F32, tag="msb")
                nc.scalar.copy(msb[:], ps[:])
                ps2 = psumt.tile([128, SCHUNK], F32, tag="tp")
                nc.tensor.transpose(
                    out=ps2[:], in_=msb[:], identity=ident[:]
                )
                osb = outp.tile([128, H], F32, tag="osb")
                nc.scalar.copy(osb[:], ps2[:])
                nc.sync.dma_start(
                    out_d[t * SCHUNK:(t + 1) * SCHUNK, :], osb[:]
                )

    nc.compile()
    return nc


_CACHE: dict[int, object] = {}


def _get_program(S: int):
    if S not in _CACHE:
        _CACHE[S] = build_program(S)
    return _CACHE[S]


def kernel(x, h, g, neighbor_index, neighbor_mask, pos_table, Wn):
    """Full inputs in, full output out. x and g are unused by the math
    (g only provides the zero row shape; x is unused in the reference)."""
    h = np.asarray(h)
    idx = np.asarray(neighbor_index)
    msk = np.asarray(neighbor_mask)
    pos = np.ascontiguousarray(np.asarray(pos_table), dtype=np.float32)
    wn = np.ascontiguousarray(np.asarray(Wn), dtype=np.float32)
    b, s, n = idx.shape
    assert (b, n) == (B, N) and h.shape == (B, s, H)

    nc = _get_program(s)
    in_maps = [
        {
            "h": np.ascontiguousarray(h[c], dtype=np.float32),
            "idx": np.ascontiguousarray(idx[c], dtype=np.int32),
            "msk": np.ascontiguousarray(msk[c], dtype=np.int32),
            "pos": pos,
            "wn": wn,
        }
        for c in range(B)
    ]
    res = run_bass_kernel_spmd(nc, in_maps, core_ids=list(range(B)))
    return np.stack([res.results[c]["out"] for c in range(B)], axis=0)
